# revision 12
# baseline (speedup 1.0000x reference)
"""Multi-head self-attention (nn_CrossAttention, B=2 S=2048 D=1024 H=16 Dh=64)
on 8 Trainium2 NeuronCores.

v2: sequence-parallel I/O + tensor-parallel attention. Host ships only
natural-layout slices (zero host-side transposes / reductions):
  core c receives x[512c:512(c+1)] (2 MB), row slices Wq/Wk/Wv[128c:128(c+1)]
  and Wo[128c:128(c+1)] (0.5 MB each), bo. Everything else happens on-device:

  setup:   PE-transpose own x slice -> xT_own, AllGather -> xT full [1024,4096]
           PE-transpose weight slices; AllGather Wo^T (each core needs full
           Wo^T for its token slice of the output projection).
  phase 1: qT,kT,vT = W @ x.T for this core's 2 heads (128-wide inner slice),
           streamed over 512-token chunks; v transposed to [token, dh] via PE,
           ones-column appended per j-tile (v_ext).
  phase 2: per (batch, 512-query chunk): scores^T blocks [keys, queries] via
           row-packed K=64 matmul pairs (two heads in PE row groups 0-63 /
           64-127), exp on ACT (max-subtraction skipped: |scaled scores| <
           ~10), accumulated [v | 1]^T @ P^T matmul giving out^T and softmax
           sums; normalize via reciprocal + partition_broadcast + DVE mul.
           Normalized chunk -> DRAM.
  a2a:     AllToAll redistributes out^T from (own heads, all tokens) to
           (all heads, own tokens) -- 1.75 MB/core on the wire.
  phase 3: y[own 512 tokens, 1024] = out^T.T @ Wo^T + bo, with the bias row
           folded in as a K=1 ones matmul. DMA 2 MB to the core's output.

Output is [512, 1024] per core; concatenated across cores it is exactly
y.reshape(4096, 1024) -- no host-side gather math.
"""

import sys

if "/opt/trn_rl_repo" not in sys.path:
    sys.path.insert(0, "/opt/trn_rl_repo")

import numpy as np

B, S, D = 2, 2048, 1024
H, DH = 16, 64
SCALE = DH**-0.5
N = B * S  # 4096 tokens total
MPC = 128  # inner-dim slice per core (2 heads)
NCORES = 8
TPC = N // NCORES  # 512 tokens per core
CH = 512  # token chunk
NCH = N // CH  # 8
DT = D // 128  # 8 contraction tiles
JT = S // 128  # 16 key tiles per batch
IC = S // CH  # 4 query chunks per batch

_cache = {}


def _build_nc():
    from contextlib import ExitStack

    import concourse.bacc as bacc
    import concourse.bass as bass
    import concourse.tile as tile
    from concourse import mybir
    from concourse.masks import make_identity

    F32 = mybir.dt.float32
    F32R = mybir.dt.float32r
    Exp = mybir.ActivationFunctionType.Exp

    nc = bacc.Bacc(
        "TRN2",
        target_bir_lowering=False,
        debug=False,
        num_devices=NCORES,
        enable_partition_id=False,
    )
    x_d = nc.dram_tensor("x", [TPC, D], F32, kind="ExternalInput").ap()
    wq_d = nc.dram_tensor("wq", [MPC, D], F32, kind="ExternalInput").ap()
    wk_d = nc.dram_tensor("wk", [MPC, D], F32, kind="ExternalInput").ap()
    wv_d = nc.dram_tensor("wv", [MPC, D], F32, kind="ExternalInput").ap()
    wo_d = nc.dram_tensor("wo", [MPC, D], F32, kind="ExternalInput").ap()
    bo_d = nc.dram_tensor("bo", [1, D], F32R, kind="ExternalInput").ap()
    ones_d = nc.dram_tensor("ones", [128, 2], F32R, kind="ExternalInput").ap()
    y_d = nc.dram_tensor("y", [TPC, D], F32, kind="ExternalOutput").ap()

    RG = [list(range(NCORES))]

    with tile.TileContext(nc) as tc, ExitStack() as ctx:
        const = ctx.enter_context(tc.tile_pool(name="const", bufs=1))
        big = ctx.enter_context(tc.tile_pool(name="big", bufs=1))
        natp = ctx.enter_context(tc.tile_pool(name="natp", bufs=2))
        xtp = ctx.enter_context(tc.tile_pool(name="xtp", bufs=2))
        vtmp = ctx.enter_context(tc.tile_pool(name="vtmp", bufs=1))
        ptp = ctx.enter_context(tc.tile_pool(name="ptp", bufs=3))
        misc = ctx.enter_context(tc.tile_pool(name="misc", bufs=3))
        otp = ctx.enter_context(tc.tile_pool(name="otp", bufs=2))
        ysbp = ctx.enter_context(tc.tile_pool(name="ysbp", bufs=1))
        ps = ctx.enter_context(tc.tile_pool(name="ps", bufs=1, space="PSUM"))
        dram = ctx.enter_context(tc.tile_pool(name="dram", bufs=1, space="DRAM"))

        ident = const.tile([128, 128], F32)
        make_identity(nc, ident)

        # DRAM bounce buffers for collectives. x^T and Wo^T share one
        # AllGather (row d carries [512 xT cols | 128 woT cols]) -- one
        # launch overhead, and the bigger message gets better effective BW.
        CW = TPC + MPC  # 640 combined columns
        comb_own_d = dram.tile([D, CW], F32R, tag="comb_own")
        comb_all_d = dram.tile([NCORES, D, CW], F32R, tag="comb_all",
                               addr_space="Shared")
        outT_own_d = dram.tile([NCH, 128, CH], F32R, tag="outT_own")
        outT_all_d = dram.tile([NCORES, 128, CH], F32R, tag="outT_all")

        # ---- setup: transpose own x slice + Wo slice, one AllGather ------
        xT_own = xtp.tile([128, DT, TPC], F32R, tag="xt")
        for tt in range(TPC // 128):
            xn = natp.tile([128, D], F32, tag="xn")
            nc.sync.dma_start(xn, x_d[tt * 128 : (tt + 1) * 128, :])
            for dt in range(DT):
                tp = ps.tile([128, CH], F32, tag="st", bufs=2)
                nc.tensor.transpose(tp[:, 0:128], xn[:, dt * 128 : (dt + 1) * 128], ident)
                nc.vector.tensor_copy(
                    xT_own[:, dt, tt * 128 : (tt + 1) * 128], tp[:, 0:128]
                )
        woT_own = const.tile([128, DT, MPC], F32R)
        wn = natp.tile([128, D], F32, tag="wn")
        nc.sync.dma_start(wn, wo_d)
        for dt in range(DT):
            tp = ps.tile([128, CH], F32, tag="st", bufs=2)
            nc.tensor.transpose(tp[:, 0:128], wn[:, dt * 128 : (dt + 1) * 128], ident)
            nc.vector.tensor_copy(woT_own[:, dt, :], tp[:, 0:128])
        comb_r = comb_own_d[:].rearrange("(t p) n -> p t n", p=128)
        nc.sync.dma_start(comb_r[:, :, 0:TPC], xT_own)
        nc.sync.dma_start(comb_r[:, :, TPC:CW], woT_own)
        nc.gpsimd.collective_compute(
            "AllGather", mybir.AluOpType.bypass, replica_groups=RG,
            ins=[comb_own_d[:].opt()], outs=[comb_all_d[:].opt()],
        )

        # ---- setup: transpose q/k/v weight slices (own heads) ------------
        wqT = const.tile([128, DT, MPC], F32R)
        wkT = const.tile([128, DT, MPC], F32R)
        wvT = const.tile([128, DT, MPC], F32R)
        for w_src, w_dst in ((wq_d, wqT), (wk_d, wkT), (wv_d, wvT)):
            wn = natp.tile([128, D], F32, tag="wn")
            nc.sync.dma_start(wn, w_src)
            for dt in range(DT):
                tp = ps.tile([128, CH], F32, tag="st", bufs=2)
                nc.tensor.transpose(tp[:, 0:128], wn[:, dt * 128 : (dt + 1) * 128], ident)
                nc.vector.tensor_copy(w_dst[:, dt, :], tp[:, 0:128])
        # full Wo^T [inner, dout] -> SBUF [p, itile, dout] (dout = r*128+dc)
        woT = big.tile([128, DT, D], F32R)
        for r in range(NCORES):
            nc.sync.dma_start(
                woT[:, :, r * MPC : (r + 1) * MPC],
                comb_all_d[r].rearrange("(t p) n -> p t n", p=128)[:, :, TPC:CW],
            )

        qT = big.tile([128, N], F32R)
        kT = big.tile([128, N], F32R)
        v_ext = big.tile([128, 2 * JT, 130], F32R)

        # ones columns of v_ext (64 for head A, 129 for head B)
        for col in (64, 129):
            src = bass.AP(
                tensor=ones_d.tensor, offset=0, ap=[[2, 128], [0, 2 * JT], [1, 1]]
            )
            nc.sync.dma_start(v_ext[:, :, col : col + 1], src)
        # ones row + bias row for the phase-3 bias matmul
        ones_row = const.tile([1, 128], F32R)
        src = bass.AP(tensor=ones_d.tensor, offset=0, ap=[[1, 1], [1, 128]])
        nc.sync.dma_start(ones_row, src)
        bias_row = const.tile([1, D], F32R)
        nc.sync.dma_start(bias_row, bo_d)

        # ---- phase 1: projections (own heads, all tokens) ----------------
        for ch in range(NCH):
            nsl = slice(ch * CH, (ch + 1) * CH)
            xt = xtp.tile([128, DT, CH], F32R, tag="xt")
            nc.sync.dma_start(
                xt, comb_all_d[ch].rearrange("(t p) n -> p t n", p=128)[:, :, 0:TPC]
            )
            for wT, dst in ((wqT, qT), (wkT, kT)):
                pps = ps.tile([128, CH], F32, tag="proj", bufs=2)
                for t in range(DT):
                    nc.tensor.matmul(
                        pps, wT[:, t, :], xt[:, t, :],
                        start=(t == 0), stop=(t == DT - 1),
                    )
                nc.vector.tensor_copy(dst[:, nsl], pps)
            vps = ps.tile([128, CH], F32, tag="proj", bufs=2)
            for t in range(DT):
                nc.tensor.matmul(
                    vps, wvT[:, t, :], xt[:, t, :],
                    start=(t == 0), stop=(t == DT - 1),
                )
            vtm = vtmp.tile([128, CH], F32, tag="vtm")
            nc.vector.tensor_copy(vtm, vps)
            for sub in range(CH // 128):
                jg = ch * (CH // 128) + sub
                tp = ps.tile([128, CH], F32, tag="st", bufs=2)
                nc.tensor.transpose(
                    tp[:, 0:128], vtm[:, sub * 128 : (sub + 1) * 128], ident
                )
                nc.vector.tensor_copy(v_ext[:, jg, 0:64], tp[:, 0:64])
                nc.vector.tensor_copy(v_ext[:, jg, 65:129], tp[:, 64:128])

        # ---- phase 2: attention (own heads) ------------------------------
        for b in range(B):
            for ic in range(IC):
                ch = b * IC + ic
                isl = slice(b * S + ic * CH, b * S + (ic + 1) * CH)
                avA = ps.tile([65, CH], F32, tag="av", bufs=2)
                avB = ps.tile([65, CH], F32, tag="av", bufs=2)
                for jp in range(JT // 2):
                    # two j-tiles share one [128, 1024] psum per head so the
                    # exp runs as one ACT instruction (amortizes ~352-cycle
                    # per-instruction overhead)
                    stA = ps.tile([128, 2 * CH], F32, tag="st", bufs=2)
                    stB = ps.tile([128, 2 * CH], F32, tag="st", bufs=2)
                    for sub in range(2):
                        jt = jp * 2 + sub
                        jsl = slice(b * S + jt * 128, b * S + (jt + 1) * 128)
                        ssl = slice(sub * CH, (sub + 1) * CH)
                        nc.tensor.matmul(
                            stA[:, ssl], kT[0:64, jsl], qT[0:64, isl],
                            start=True, stop=True,
                        )
                        nc.tensor.matmul(
                            stB[:, ssl], kT[64:128, jsl], qT[64:128, isl],
                            start=True, stop=True,
                        )
                    ptA = ptp.tile([128, 2 * CH], F32R, tag="pt")
                    ptB = ptp.tile([128, 2 * CH], F32R, tag="pt")
                    nc.scalar.activation(ptA, stA, Exp, scale=SCALE)
                    nc.scalar.activation(ptB, stB, Exp, scale=SCALE)
                    for sub in range(2):
                        jt = jp * 2 + sub
                        jg = b * JT + jt
                        ssl = slice(sub * CH, (sub + 1) * CH)
                        nc.tensor.matmul(
                            avA, v_ext[:, jg, 0:65], ptA[:, ssl],
                            start=(jt == 0), stop=(jt == JT - 1),
                        )
                        nc.tensor.matmul(
                            avB, v_ext[:, jg, 65:130], ptB[:, ssl],
                            start=(jt == 0), stop=(jt == JT - 1),
                        )
                ot = otp.tile([128, CH], F32R, tag="ot")
                for h, av in ((0, avA), (1, avB)):
                    lrow = misc.tile([1, CH], F32, tag="lrow")
                    nc.vector.tensor_copy(lrow, av[64:65, :])
                    rrow = misc.tile([1, CH], F32, tag="rrow")
                    nc.vector.reciprocal(rrow, lrow)
                    rbs = misc.tile([64, CH], F32, tag="rbs")
                    nc.gpsimd.partition_broadcast(rbs, rrow)
                    nc.vector.tensor_mul(
                        ot[h * 64 : (h + 1) * 64, :], av[0:64, :], rbs
                    )
                nc.sync.dma_start(outT_own_d[ch], ot)

        # ---- AllToAll: (own heads, all tokens) -> (all heads, own tokens)
        nc.gpsimd.collective_compute(
            "AllToAll", mybir.AluOpType.bypass, replica_groups=RG,
            ins=[outT_own_d[:].opt()], outs=[outT_all_d[:].opt()],
        )
        outT = big.tile([128, DT, CH], F32R)
        for r in range(NCORES):
            nc.sync.dma_start(outT[:, r, :], outT_all_d[r])

        # ---- phase 3: output projection for own 512 tokens ---------------
        for nt in range(TPC // 128):
            ysb = ysbp.tile([128, D], F32, tag="ysb")
            for c2 in range(D // CH):
                dsl = slice(c2 * CH, (c2 + 1) * CH)
                yps = ps.tile([128, CH], F32, tag="proj", bufs=2)
                nc.tensor.matmul(
                    yps, ones_row, bias_row[0:1, dsl], start=True, stop=False
                )
                for r in range(NCORES):
                    nc.tensor.matmul(
                        yps,
                        outT[:, r, nt * 128 : (nt + 1) * 128],
                        woT[:, r, dsl],
                        start=False, stop=(r == NCORES - 1),
                    )
                nc.vector.tensor_copy(ysb[:, dsl], yps)
            nc.sync.dma_start(y_d[nt * 128 : (nt + 1) * 128, :], ysb)

    nc.compile()
    return nc


def _get_nc():
    if "nc" not in _cache:
        _cache["nc"] = _build_nc()
    return _cache["nc"]


def make_in_maps(x, Wq, Wk, Wv, Wo, bo=None):
    """Per-core input dicts (views -- no host copies)."""
    x = np.asarray(x, dtype=np.float32).reshape(N, D)
    Wq = np.asarray(Wq, np.float32)
    Wk = np.asarray(Wk, np.float32)
    Wv = np.asarray(Wv, np.float32)
    Wo = np.asarray(Wo, np.float32)
    bo = (
        np.zeros((1, D), np.float32)
        if bo is None
        else np.asarray(bo, np.float32).reshape(1, D)
    )
    ones = np.ones((128, 2), dtype=np.float32)
    in_maps = []
    for c in range(NCORES):
        ms = slice(c * MPC, (c + 1) * MPC)
        in_maps.append(
            {
                "x": x[c * TPC : (c + 1) * TPC],
                "wq": Wq[ms],
                "wk": Wk[ms],
                "wv": Wv[ms],
                "wo": Wo[ms],
                "bo": bo,
                "ones": ones,
            }
        )
    return in_maps


def _get_runner():
    """Cached jitted 8-core runner (mirrors bass2jax.run_bass_via_pjrt's
    multi-core path so repeated calls reuse the compiled NEFF)."""
    if "runner" in _cache:
        return _cache["runner"]

    import jax
    from jax.experimental.shard_map import shard_map
    from jax.sharding import Mesh, NamedSharding, PartitionSpec

    import concourse.mybir as mybir
    from concourse import bass2jax

    bass2jax.install_neuronx_cc_hook()
    nc = _get_nc()

    in_names, out_names, out_avals, zero_outs = [], [], [], []
    for alloc in nc.m.functions[0].allocations:
        if not isinstance(alloc, mybir.MemoryLocationSet):
            continue
        name = alloc.memorylocations[0].name
        if alloc.kind == "ExternalInput":
            in_names.append(name)
        elif alloc.kind == "ExternalOutput":
            out_names.append(name)
            shape = tuple(alloc.tensor_shape)
            dtype = mybir.dt.np(alloc.dtype)
            out_avals.append(jax.core.ShapedArray(shape, dtype))
            zero_outs.append(np.zeros(shape, dtype))
    n_params = len(in_names)
    n_outs = len(out_avals)
    all_in_names = in_names + out_names

    def _body(*args):
        outs = bass2jax._bass_exec_p.bind(
            *args,
            out_avals=tuple(out_avals),
            in_names=tuple(all_in_names),
            out_names=tuple(out_names),
            lowering_input_output_aliases=(),
            sim_require_finite=True,
            sim_require_nnan=True,
            nc=nc,
        )
        return tuple(outs)

    devices = jax.devices()[:NCORES]
    mesh = Mesh(np.asarray(devices), ("core",))
    donate = tuple(range(n_params, n_params + n_outs))
    sharded = jax.jit(
        shard_map(
            _body,
            mesh=mesh,
            in_specs=(PartitionSpec("core"),) * (n_params + n_outs),
            out_specs=(PartitionSpec("core"),) * n_outs,
            check_rep=False,
        ),
        donate_argnums=donate,
        keep_unused=True,
    )

    # fresh donated output buffers per call
    sh = NamedSharding(mesh, PartitionSpec("core"))
    zshapes = tuple((NCORES * z.shape[0], *z.shape[1:]) for z in zero_outs)
    zdtypes = tuple(z.dtype for z in zero_outs)

    def _make_zeros():
        return tuple(
            jax.device_put(np.zeros(s, d), sh) for s, d in zip(zshapes, zdtypes)
        )

    _cache["runner"] = (sharded, in_names, out_names, zero_outs, mesh, _make_zeros)
    return _cache["runner"]


def run_cores(in_maps):
    """Run the 8-core NEFF, return list of per-core output dicts."""
    sharded, in_names, out_names, zero_outs, _, make_zeros = _get_runner()
    concat_in = [
        np.concatenate([np.asarray(m[name]) for m in in_maps], axis=0)
        for name in in_names
    ]
    out_arrs = sharded(*concat_in, *make_zeros())
    per_core = []
    for c in range(NCORES):
        per_core.append(
            {
                name: np.asarray(out_arrs[i]).reshape(
                    NCORES, out_arrs[i].shape[0] // NCORES, *out_arrs[i].shape[1:]
                )[c]
                for i, name in enumerate(out_names)
            }
        )
    return per_core


def kernel(x, Wq, Wk, Wv, Wo, bo):
    sharded, in_names, out_names, zero_outs, mesh, make_zeros = _get_runner()
    x = np.asarray(x, dtype=np.float32).reshape(N, D)
    full_in = {
        "x": x,
        "wq": np.asarray(Wq, np.float32),
        "wk": np.asarray(Wk, np.float32),
        "wv": np.asarray(Wv, np.float32),
        "wo": np.asarray(Wo, np.float32),
        "bo": np.ascontiguousarray(
            np.broadcast_to(np.asarray(bo, np.float32).reshape(1, D), (NCORES, D))
        ),
        "ones": np.ones((NCORES * 128, 2), dtype=np.float32),
    }
    concat_in = [full_in[name] for name in in_names]
    out_arrs = sharded(*concat_in, *make_zeros())
    y = np.asarray(out_arrs[out_names.index("y")])
    return y.reshape(B, S, D)
